# revision 23
# baseline (speedup 1.0000x reference)
"""Batched 20x20 SPD covariance-matrix inversion on 8 Trainium2 NeuronCores.

For each of 131072 batches: build C = exp(-1.5 * pairwise_dist(pos)) + 0.01*I
from 20 2-D points, return C^{-1}.

Strategy (per core, data-parallel over batch):
 - batch-major layout: each of 128 SBUF partitions holds Mg matrices' full
   20x20 (400 fp32) in the free dim; N_STREAMS independent streams.
 - symmetric sweep operator (Gauss-Jordan preserving symmetry): only the
   upper triangle is updated each pivot, covered by 4 row-band rectangles.
 - engine split: the rank-1 updates (2 tensor-tensor passes per element)
   are split row-wise between DVE (tensor_tensor) and GPSIMD
   (scalar_tensor_tensor); ACT does the pivot-column gathers, the
   cov-build square/sqrt/exp, and the final negated mirror of the upper
   triangle into the lower.
 - pivot row/col are written AFTER the rank-1 update (which is allowed to
   corrupt them), so the pivot column never needs zeroing.
 - ACTIVE streams are software-pipelined round-robin at pivot granularity:
   one stream's serial pivot prefix (gather -> reciprocal -> cr scale)
   hides under the other's rank-1 work; cov builds and finalizes of
   adjacent streams overlap sweeps the same way. All pos DMAs are
   prefetched at kernel start.
"""

import numpy as np

import concourse.bass as bass  # noqa: F401  (registers engine APIs)
import concourse.tile as tile
from concourse import bacc, mybir
from concourse.bass_utils import run_bass_kernel_spmd

N = 20                  # matrix dim
D = 2                   # coord dim
PHI = 1.5
TAU = 0.01
P = 128                 # SBUF partitions
N_CORES = 8
B_TOTAL = 131072
B_CORE = B_TOTAL // N_CORES   # 16384

F32 = mybir.dt.float32
AF = mybir.ActivationFunctionType
OP = mybir.AluOpType

# Upper-triangle rectangle cover: rows [r0,r1) x cols [r0,N)
RECTS = [(0, 5), (5, 10), (10, 15), (15, 20)]

# --- engine-assignment knobs (autotuned via CoreSim) -----------------------
# per rect: how many of its rows (from the top) go to GPSIMD for the rank-1
# update; the rest go to DVE.
GP_ROWS = [5, 1, 0, 5]
# per rect: cov-build tensor-tensor ops (dx, dy, add) engine: "v" DVE, "g" GP
COV_ENG = ["g", "g", "v", "v"]
CR_ENG = "g"            # cr = c * r:  "v" DVE tensor_mul, "g" GP stt
MIRROR_ENG = "a"        # "a" ACT copy(scale=-1) pre-negate, "v" DVE post
GATHER_SCALAR = True    # pivot-column gather on ACT (else DVE)
PIVOT_COPY_ENG = "a"    # pivot row/col <- cr copies: "v" DVE, "a" ACT, "g" GP
N_STREAMS = 8           # independent m-slices (Mg = B_CORE/P/N_STREAMS)
ACTIVE = 2              # streams pipelined concurrently
STAGGER = 4             # yields to prime stream 0 before starting stream 1
COV_MERGED = False       # cov: one 2-coord sub (needs 2E dd tile) vs dx/dy
DIAG_ENG = "v"          # per-pivot diag<- -r and diag TAU add: "a" ACT, "v" DVE


def _gp_mul(nc, out, a, b):
    """out = a * b on GPSIMD (plain TensorTensor: >=4D APs are BIR-legal,
    unlike ScalarTensorTensor which the BIR verifier caps at 3D)."""
    nc.gpsimd.tensor_tensor(out, a, b, OP.mult)


def _gp_sub(nc, out, a, b):
    """out = a - b on GPSIMD."""
    nc.gpsimd.tensor_tensor(out, a, b, OP.subtract)


def _gp_add(nc, out, a, b):
    nc.gpsimd.tensor_tensor(out, a, b, OP.add)


def emit_kernel(tc, pos_ap, out_ap, b_core, n_streams):
    """Emit the per-core program. pos: [b_core, 40] f32, out: [b_core, 400] f32."""
    nc = tc.nc
    Mg = b_core // (P * n_streams)
    assert b_core == P * Mg * n_streams

    pos_r = pos_ap.rearrange("(p s m) f -> p s (m f)", p=P, s=n_streams)
    out_r = out_ap.rearrange("(p s m) f -> p s (m f)", p=P, s=n_streams)
    big_bufs = 2 if (ACTIVE <= 2 and Mg < 32) else 1

    with (
        tc.tile_pool(name="pos", bufs=1) as pos_pool,
        tc.tile_pool(name="A", bufs=big_bufs) as a_pool,
        tc.tile_pool(name="cov", bufs=big_bufs) as cov_pool,
        tc.tile_pool(name="rect", bufs=1) as rect_pool,
        tc.tile_pool(name="grect", bufs=1) as grect_pool,
        tc.tile_pool(name="small", bufs=2) as small_pool,
    ):
        # prefetch every stream's positions up front
        pos_tiles = []
        for s in range(n_streams):
            pos_t = pos_pool.tile([P, Mg * N * D], F32, tag=f"pos{s}")
            nc.sync.dma_start(pos_t[:, :], pos_r[:, s, :])
            pos_tiles.append(pos_t)

        def stream_gen(s):
            par = s % ACTIVE
            posv = pos_tiles[s][:, :].rearrange(
                "p (m i d) -> p m i d", m=Mg, i=N
            )
            A = a_pool.tile([P, Mg * N * N], F32, tag=f"A{par}")
            A4 = A[:, :].rearrange("p (m i j) -> p m i j", m=Mg, i=N)

            # ---- covariance build over the upper rect cover ----
            for ri, (r0, r1) in enumerate(RECTS):
                nr, ncl = r1 - r0, N - r0
                reg = A4[:, :, r0:r1, r0:]
                sub_a = (
                    (lambda o, a, b: _gp_sub(nc, o, a, b))
                    if COV_ENG[ri] == "g"
                    else nc.vector.tensor_sub
                )
                add_a = (
                    (lambda o, a, b: _gp_add(nc, o, a, b))
                    if COV_ENG[ri] == "g"
                    else nc.vector.tensor_add
                )
                if COV_MERGED:
                    # dd[m,i,j,:] = p[i,:] - p[j,:] in ONE sub (both
                    # coords), square on ACT, strided add -> d^2 in A.
                    pi = (
                        posv[:, :, r0:r1, :]
                        .unsqueeze(3)
                        .broadcast_to([P, Mg, nr, ncl, D])
                    )
                    pj = (
                        posv[:, :, r0:, :]
                        .unsqueeze(2)
                        .broadcast_to([P, Mg, nr, ncl, D])
                    )
                    dd = cov_pool.tile(
                        [P, Mg * nr * ncl * D], F32, tag=f"dd{par}"
                    )
                    ddv = dd[:, :].rearrange(
                        "p (m i j d) -> p m i j d", m=Mg, i=nr, j=ncl
                    )
                    sub_a(ddv, pi, pj)
                    nc.scalar.square(dd[:, :], dd[:, :])
                    add_a(reg, ddv[:, :, :, :, 0], ddv[:, :, :, :, 1])
                else:
                    # dx into A (in-place square), dy in an E-sized tmp
                    xi = (
                        posv[:, :, r0:r1, 0]
                        .unsqueeze(3)
                        .broadcast_to([P, Mg, nr, ncl])
                    )
                    xj = (
                        posv[:, :, r0:, 0]
                        .unsqueeze(2)
                        .broadcast_to([P, Mg, nr, ncl])
                    )
                    yi = (
                        posv[:, :, r0:r1, 1]
                        .unsqueeze(3)
                        .broadcast_to([P, Mg, nr, ncl])
                    )
                    yj = (
                        posv[:, :, r0:, 1]
                        .unsqueeze(2)
                        .broadcast_to([P, Mg, nr, ncl])
                    )
                    dy = cov_pool.tile(
                        [P, Mg * nr * ncl], F32, tag=f"dd{par}"
                    )
                    dyv = dy[:, :].rearrange(
                        "p (m i j) -> p m i j", m=Mg, i=nr
                    )
                    sub_a(reg, xi, xj)
                    nc.scalar.square(reg, reg)
                    sub_a(dyv, yi, yj)
                    nc.scalar.square(dyv, dyv)
                    add_a(reg, reg, dyv)
                nc.scalar.sqrt(reg, reg)
                nc.scalar.activation(reg, reg, AF.Exp, scale=-PHI)
                yield

            Av = A[:, :].rearrange("p (m x) -> p m x", m=Mg)
            diag = Av[:, :, 0 : N * N : N + 1]
            nc.vector.tensor_scalar_add(diag, diag, TAU)

            # ---- sweep all 20 pivots ----
            for k in range(N):
                cK = small_pool.tile([P, Mg * N], F32, tag=f"c{par}")
                crK = small_pool.tile([P, Mg * N], F32, tag=f"cr{par}")
                rK = small_pool.tile([P, Mg], F32, tag=f"r{par}")
                c3 = cK[:, :].rearrange("p (m i) -> p m i", m=Mg)
                cr3 = crK[:, :].rearrange("p (m i) -> p m i", m=Mg)

                # gather pivot column from upper storage (ACT engine)
                gat = (
                    nc.scalar.copy if GATHER_SCALAR else nc.vector.tensor_copy
                )
                if k:
                    gat(c3[:, :, :k], A4[:, :, :k, k])
                gat(c3[:, :, k:], A4[:, :, k, k:])
                nc.vector.reciprocal(rK[:, :], c3[:, :, k])
                rb = rK[:, :].unsqueeze(2).broadcast_to([P, Mg, N])
                if CR_ENG == "g":
                    _gp_mul(nc, cr3, c3, rb)
                else:
                    nc.vector.tensor_mul(cr3, c3, rb)
                # rank-1 update of the upper triangle (rect cover),
                # row-split between GPSIMD (top rows) and DVE (rest).
                # c3[k] != 0 and cr3[k] = 1, so the update corrupts the
                # pivot row/col inside the cover; they are rewritten
                # below, and sub-diagonal positions are never read.
                for ri, (r0, r1) in enumerate(RECTS):
                    ncl = N - r0
                    gsplit = r0 + GP_ROWS[ri]
                    for eng, a, b in (("g", r0, gsplit), ("v", gsplit, r1)):
                        nr = b - a
                        if nr <= 0:
                            continue
                        pool = grect_pool if eng == "g" else rect_pool
                        tmp = pool.tile(
                            [P, Mg * nr * ncl], F32, tag=f"t{eng}{par}"
                        )
                        tv = tmp[:, :].rearrange(
                            "p (m i j) -> p m i j", m=Mg, i=nr
                        )
                        cb = (
                            c3[:, :, a:b]
                            .unsqueeze(3)
                            .broadcast_to([P, Mg, nr, ncl])
                        )
                        crb = (
                            cr3[:, :, r0:]
                            .unsqueeze(2)
                            .broadcast_to([P, Mg, nr, ncl])
                        )
                        reg = A4[:, :, a:b, r0:]
                        last = k == N - 1
                        if eng == "g":
                            _gp_mul(nc, tv, cb, crb)
                            if last:
                                # fold the final negation into the last
                                # pivot: reg <- tv - reg = -(reg - tv)
                                _gp_sub(nc, reg, tv, reg)
                            else:
                                _gp_sub(nc, reg, reg, tv)
                        else:
                            nc.vector.tensor_mul(tv, cb, crb)
                            if last:
                                nc.vector.tensor_sub(reg, tv, reg)
                            else:
                                nc.vector.tensor_sub(reg, reg, tv)
                # pivot row/col (upper parts) <- cr; diag <- -r
                # (the last pivot writes negated values: the whole rect
                # cover holds -result after its reverse subtract)
                if k < N - 1:
                    pcopy = {
                        "v": nc.vector.tensor_copy,
                        "a": nc.scalar.copy,
                        "g": nc.gpsimd.tensor_copy,
                    }[PIVOT_COPY_ENG]
                    if k:
                        pcopy(A4[:, :, :k, k], cr3[:, :, :k])
                    pcopy(A4[:, :, k, k + 1 :], cr3[:, :, k + 1 :])
                    if DIAG_ENG == "a":
                        nc.scalar.mul(A4[:, :, k, k], rK[:, :], -1.0)
                    else:
                        nc.vector.tensor_scalar_mul(
                            A4[:, :, k, k], rK[:, :], -1.0
                        )
                else:
                    if PIVOT_COPY_ENG == "a":
                        nc.scalar.mul(A4[:, :, :k, k], cr3[:, :, :k], -1.0)
                    else:
                        nc.vector.tensor_scalar_mul(
                            A4[:, :, :k, k], cr3[:, :, :k], -1.0
                        )
                    nc.vector.tensor_copy(A4[:, :, k, k], rK[:, :])
                yield

            # ---- finalize: mirror upper -> lower (values already negated)
            mcopy = (
                nc.scalar.copy if MIRROR_ENG == "a" else nc.vector.tensor_copy
            )
            for i in range(N - 1):
                mcopy(A4[:, :, i + 1 :, i], A4[:, :, i, i + 1 :])
            yield

            nc.sync.dma_start(out_r[:, s, :], A[:, :])

        pending = list(range(n_streams))
        active = [stream_gen(pending.pop(0))]
        # prime the first stream so concurrent streams stay phase-offset
        for _ in range(STAGGER):
            next(active[0])
        while pending or active:
            while len(active) < ACTIVE and pending:
                active.append(stream_gen(pending.pop(0)))
            for gen in list(active):
                try:
                    next(gen)
                except StopIteration:
                    active.remove(gen)


_CACHE = {}


def build_nc(b_core=B_CORE, n_streams=None, num_devices=N_CORES):
    if n_streams is None:
        n_streams = N_STREAMS
    key = (b_core, n_streams, num_devices)
    if key in _CACHE:
        return _CACHE[key]
    nc = bacc.Bacc(
        "TRN2", target_bir_lowering=False, debug=False, num_devices=num_devices
    )
    pos_d = nc.dram_tensor("pos", [b_core, N * D], F32, kind="ExternalInput")
    out_d = nc.dram_tensor("out", [b_core, N * N], F32, kind="ExternalOutput")
    with tile.TileContext(nc) as tc:
        emit_kernel(tc, pos_d.ap(), out_d.ap(), b_core, n_streams)
    nc.compile()
    _CACHE[key] = nc
    return nc


def run(pos_full, b_core=B_CORE, n_streams=None, n_cores=N_CORES, **kw):
    """pos_full: [n_cores*b_core, 20, 2] f32 -> [n_cores*b_core, 20, 20] f32."""
    nc = build_nc(b_core, n_streams, n_cores)
    flat = np.ascontiguousarray(
        np.asarray(pos_full, dtype=np.float32).reshape(-1, N * D)
    )
    in_maps = [
        {"pos": flat[i * b_core : (i + 1) * b_core]} for i in range(n_cores)
    ]
    res = run_bass_kernel_spmd(nc, in_maps, core_ids=list(range(n_cores)), **kw)
    out = np.concatenate([r["out"] for r in res.results], axis=0)
    return out.reshape(-1, N, N), res


def kernel(neighbor_positions, edge_list=None):
    out, _ = run(neighbor_positions)
    return out


# revision 35
# speedup vs baseline: 1.0282x; 1.0282x over previous
"""Batched 20x20 SPD covariance-matrix inversion on 8 Trainium2 NeuronCores.

For each of 131072 batches: build C = exp(-1.5 * pairwise_dist(pos)) + 0.01*I
from 20 2-D points, return C^{-1}.

Strategy (per core, data-parallel over batch):
 - batch-major layout: each of 128 SBUF partitions holds Mg matrices' full
   20x20 (400 fp32) in the free dim; N_STREAMS independent streams.
 - symmetric sweep operator (Gauss-Jordan preserving symmetry): only the
   upper triangle is updated each pivot, covered by 4 row-band rectangles.
 - engine split: the rank-1 updates (2 tensor-tensor passes per element)
   are split row-wise between DVE (tensor_tensor) and GPSIMD
   (scalar_tensor_tensor); ACT does the pivot-column gathers, the
   cov-build square/sqrt/exp, and the final negated mirror of the upper
   triangle into the lower.
 - pivot row/col are written AFTER the rank-1 update (which is allowed to
   corrupt them), so the pivot column never needs zeroing.
 - ACTIVE streams are software-pipelined round-robin at pivot granularity:
   one stream's serial pivot prefix (gather -> reciprocal -> cr scale)
   hides under the other's rank-1 work; cov builds and finalizes of
   adjacent streams overlap sweeps the same way. All pos DMAs are
   prefetched at kernel start.
"""

import numpy as np

import concourse.bass as bass  # noqa: F401  (registers engine APIs)
import concourse.tile as tile
from concourse import bacc, mybir
from concourse.bass_utils import run_bass_kernel_spmd

N = 20                  # matrix dim
D = 2                   # coord dim
PHI = 1.5
TAU = 0.01
P = 128                 # SBUF partitions
N_CORES = 8
B_TOTAL = 131072
B_CORE = B_TOTAL // N_CORES   # 16384

F32 = mybir.dt.float32
AF = mybir.ActivationFunctionType
OP = mybir.AluOpType

# Upper-triangle rectangle cover: rows [r0,r1) x cols [r0,N)
RECTS = [(0, 5), (5, 10), (10, 15), (15, 20)]

# --- engine-assignment knobs (autotuned via CoreSim) -----------------------
# per rect: how many of its rows (from the top) go to GPSIMD for the rank-1
# update; the rest go to DVE.
GP_ROWS = [5, 1, 1, 5]
# per rect: cov-build tensor-tensor ops (dx, dy, add) engine: "v" DVE, "g" GP
COV_ENG = ["v", "g", "g", "v"]
CR_ENG = "s"            # cr = c * r:  "v" DVE tensor_mul, "g" GP stt
MIRROR_ENG = "a"        # "a" ACT copy(scale=-1) pre-negate, "v" DVE post
GATHER_SCALAR = True    # pivot-column gather on ACT (else DVE)
PIVOT_COPY_ENG = "a"    # pivot row/col <- cr copies: "v" DVE, "a" ACT, "g" GP
N_STREAMS = 8           # independent m-slices (Mg = B_CORE/P/N_STREAMS)
STREAM_SIZES = None     # optional per-stream m sizes (must sum to B_CORE/P)
ACTIVE = 2              # streams pipelined concurrently
STAGGER = 5             # yields to prime stream 0 before starting stream 1
COV_MERGED = False       # cov: one 2-coord sub (needs 2E dd tile) vs dx/dy
DIAG_ENG = "a"          # per-pivot diag<- -r + TAU add: "a" ACT, "v" DVE, "g" GP


def _gp_mul(nc, out, a, b):
    """out = a * b on GPSIMD (plain TensorTensor: >=4D APs are BIR-legal,
    unlike ScalarTensorTensor which the BIR verifier caps at 3D)."""
    nc.gpsimd.tensor_tensor(out, a, b, OP.mult)


def _gp_sub(nc, out, a, b):
    """out = a - b on GPSIMD."""
    nc.gpsimd.tensor_tensor(out, a, b, OP.subtract)


def _gp_add(nc, out, a, b):
    nc.gpsimd.tensor_tensor(out, a, b, OP.add)


def emit_kernel(tc, pos_ap, out_ap, b_core, n_streams):
    """Emit the per-core program. pos: [b_core, 40] f32, out: [b_core, 400] f32."""
    nc = tc.nc
    m_total = b_core // P
    if STREAM_SIZES is not None:
        sizes = list(STREAM_SIZES)
        n_streams = len(sizes)
    else:
        sizes = [m_total // n_streams] * n_streams
    assert sum(sizes) == m_total
    offs = [0]
    for sz in sizes:
        offs.append(offs[-1] + sz)

    pos_r = pos_ap.rearrange("(p m) f -> p m f", p=P)
    out_r = out_ap.rearrange("(p m) f -> p m f", p=P)
    big_bufs = 2 if (ACTIVE <= 2 and max(sizes) < 32) else 1

    with (
        tc.tile_pool(name="pos", bufs=1) as pos_pool,
        tc.tile_pool(name="A", bufs=big_bufs) as a_pool,
        tc.tile_pool(name="cov", bufs=big_bufs) as cov_pool,
        tc.tile_pool(name="rect", bufs=1) as rect_pool,
        tc.tile_pool(name="grect", bufs=1) as grect_pool,
        tc.tile_pool(name="small", bufs=2) as small_pool,
    ):
        # prefetch every stream's positions up front
        pos_tiles = []
        for s in range(n_streams):
            pos_t = pos_pool.tile([P, sizes[s] * N * D], F32, tag=f"pos{s}")
            nc.sync.dma_start(pos_t[:, :], pos_r[:, offs[s] : offs[s + 1], :])
            pos_tiles.append(pos_t)

        def stream_gen(s):
            par = s % ACTIVE
            Mg = sizes[s]
            posv = pos_tiles[s][:, :].rearrange(
                "p (m i d) -> p m i d", m=Mg, i=N
            )
            A = a_pool.tile([P, Mg * N * N], F32, tag=f"A{par}")
            A4 = A[:, :].rearrange("p (m i j) -> p m i j", m=Mg, i=N)

            # ---- covariance build over the upper rect cover ----
            for ri, (r0, r1) in enumerate(RECTS):
                nr, ncl = r1 - r0, N - r0
                reg = A4[:, :, r0:r1, r0:]
                sub_a = (
                    (lambda o, a, b: _gp_sub(nc, o, a, b))
                    if COV_ENG[ri] == "g"
                    else nc.vector.tensor_sub
                )
                add_a = (
                    (lambda o, a, b: _gp_add(nc, o, a, b))
                    if COV_ENG[ri] == "g"
                    else nc.vector.tensor_add
                )
                if COV_MERGED:
                    # dd[m,i,j,:] = p[i,:] - p[j,:] in ONE sub (both
                    # coords), square on ACT, strided add -> d^2 in A.
                    pi = (
                        posv[:, :, r0:r1, :]
                        .unsqueeze(3)
                        .broadcast_to([P, Mg, nr, ncl, D])
                    )
                    pj = (
                        posv[:, :, r0:, :]
                        .unsqueeze(2)
                        .broadcast_to([P, Mg, nr, ncl, D])
                    )
                    dd = cov_pool.tile(
                        [P, Mg * nr * ncl * D], F32, tag=f"dd{par}"
                    )
                    ddv = dd[:, :].rearrange(
                        "p (m i j d) -> p m i j d", m=Mg, i=nr, j=ncl
                    )
                    sub_a(ddv, pi, pj)
                    nc.scalar.square(dd[:, :], dd[:, :])
                    add_a(reg, ddv[:, :, :, :, 0], ddv[:, :, :, :, 1])
                else:
                    # dx into A (in-place square), dy in an E-sized tmp
                    xi = (
                        posv[:, :, r0:r1, 0]
                        .unsqueeze(3)
                        .broadcast_to([P, Mg, nr, ncl])
                    )
                    xj = (
                        posv[:, :, r0:, 0]
                        .unsqueeze(2)
                        .broadcast_to([P, Mg, nr, ncl])
                    )
                    yi = (
                        posv[:, :, r0:r1, 1]
                        .unsqueeze(3)
                        .broadcast_to([P, Mg, nr, ncl])
                    )
                    yj = (
                        posv[:, :, r0:, 1]
                        .unsqueeze(2)
                        .broadcast_to([P, Mg, nr, ncl])
                    )
                    dy = cov_pool.tile(
                        [P, Mg * nr * ncl], F32, tag=f"dd{par}"
                    )
                    dyv = dy[:, :].rearrange(
                        "p (m i j) -> p m i j", m=Mg, i=nr
                    )
                    sub_a(reg, xi, xj)
                    nc.scalar.square(reg, reg)
                    sub_a(dyv, yi, yj)
                    nc.scalar.square(dyv, dyv)
                    add_a(reg, reg, dyv)
                nc.scalar.sqrt(reg, reg)
                nc.scalar.activation(reg, reg, AF.Exp, scale=-PHI)
                yield

            Av = A[:, :].rearrange("p (m x) -> p m x", m=Mg)
            diag = Av[:, :, 0 : N * N : N + 1]
            if DIAG_ENG == "g":
                nc.gpsimd.tensor_scalar_add(diag, diag, TAU)
            else:
                nc.vector.tensor_scalar_add(diag, diag, TAU)

            # ---- sweep all 20 pivots (gather-free) ----
            # The raw pivot column/row is read straight out of A as
            # broadcast operands of the rank-1 muls (column k for rows
            # above the pivot, row k for rows below); cr (= c * 1/pivot)
            # is computed from A the same way. All muls are emitted
            # before any sub so the in-place subs (which corrupt the
            # pivot row/col: cr[k] = 1) never race the raw reads.
            # The reciprocal for pivot k+1 is issued right after the sub
            # that finalizes A[k+1,k+1], hiding it under pivot k's tail.
            rK = small_pool.tile([P, Mg], F32, tag=f"r{par}")
            nc.vector.reciprocal(rK[:, :], A4[:, :, 0, 0])
            for k in range(N):
                crK = small_pool.tile([P, Mg * N], F32, tag=f"cr{par}")
                cr3 = crK[:, :].rearrange("p (m i) -> p m i", m=Mg)
                rb = rK[:, :].unsqueeze(2).broadcast_to([P, Mg, N])

                def crmul(which, o, a, b):
                    if CR_ENG == "s" and which == 0:
                        _gp_mul(nc, o, a, b)  # split: col part on GPSIMD
                    elif CR_ENG == "g":
                        _gp_mul(nc, o, a, b)
                    else:
                        nc.vector.tensor_mul(o, a, b)

                if k:
                    crmul(0, cr3[:, :, :k], A4[:, :, :k, k], rb[:, :, :k])
                crmul(1, cr3[:, :, k:], A4[:, :, k, k:], rb[:, :, k:])

                def c_raw(a, b):
                    """Broadcast AP of raw c[a:b] read from A's storage."""
                    if b <= k + 1:  # rows at or above the pivot: column k
                        return A4[:, :, a:b, k].unsqueeze(3)
                    return A4[:, :, k, a:b].unsqueeze(3)  # below: row k

                last = k == N - 1
                subs = []
                for ri, (r0, r1) in enumerate(RECTS):
                    ncl = N - r0
                    gsplit = r0 + GP_ROWS[ri]
                    for eng, a, b in (("g", r0, gsplit), ("v", gsplit, r1)):
                        nr = b - a
                        if nr <= 0:
                            continue
                        pool = grect_pool if eng == "g" else rect_pool
                        tmp = pool.tile(
                            [P, Mg * nr * ncl], F32, tag=f"t{eng}{par}r{ri}"
                        )
                        tv = tmp[:, :].rearrange(
                            "p (m i j) -> p m i j", m=Mg, i=nr
                        )
                        crb = (
                            cr3[:, :, r0:]
                            .unsqueeze(2)
                            .broadcast_to([P, Mg, nr, ncl])
                        )
                        mul = (
                            (lambda o, x, y: _gp_mul(nc, o, x, y))
                            if eng == "g"
                            else nc.vector.tensor_mul
                        )
                        # split the band straddling the pivot: rows <= k
                        # read column k, rows > k read row k
                        if a <= k < b - 1:
                            pieces = [(a, k + 1), (k + 1, b)]
                        else:
                            pieces = [(a, b)]
                        ofs = 0
                        for (pa, pb) in pieces:
                            pn = pb - pa
                            mul(
                                tv[:, :, ofs : ofs + pn],
                                c_raw(pa, pb).broadcast_to([P, Mg, pn, ncl]),
                                crb[:, :, ofs : ofs + pn],
                            )
                            ofs += pn
                        reg = A4[:, :, a:b, r0:]
                        subs.append((eng, reg, tv))
                for eng, reg, tv in subs:
                    if eng == "g":
                        if last:
                            # fold the final negation into the last
                            # pivot: reg <- tv - reg = -(reg - tv)
                            _gp_sub(nc, reg, tv, reg)
                        else:
                            _gp_sub(nc, reg, reg, tv)
                    else:
                        if last:
                            nc.vector.tensor_sub(reg, tv, reg)
                        else:
                            nc.vector.tensor_sub(reg, reg, tv)
                # pivot row/col (upper parts) <- cr; diag <- -r
                # (the last pivot writes negated values: the whole rect
                # cover holds -result after its reverse subtract)
                if k < N - 1:
                    pcopy = {
                        "v": nc.vector.tensor_copy,
                        "a": nc.scalar.copy,
                        "g": nc.gpsimd.tensor_copy,
                    }[PIVOT_COPY_ENG]
                    if k:
                        pcopy(A4[:, :, :k, k], cr3[:, :, :k])
                    pcopy(A4[:, :, k, k + 1 :], cr3[:, :, k + 1 :])
                    if DIAG_ENG == "a":
                        nc.scalar.mul(A4[:, :, k, k], rK[:, :], -1.0)
                    elif DIAG_ENG == "g":
                        nc.gpsimd.tensor_scalar_mul(
                            A4[:, :, k, k], rK[:, :], -1.0
                        )
                    else:
                        nc.vector.tensor_scalar_mul(
                            A4[:, :, k, k], rK[:, :], -1.0
                        )
                else:
                    if PIVOT_COPY_ENG == "a":
                        nc.scalar.mul(A4[:, :, :k, k], cr3[:, :, :k], -1.0)
                    else:
                        nc.vector.tensor_scalar_mul(
                            A4[:, :, :k, k], cr3[:, :, :k], -1.0
                        )
                    nc.vector.tensor_copy(A4[:, :, k, k], rK[:, :])
                yield

            # ---- finalize: mirror upper -> lower (values already negated)
            mcopy = (
                nc.scalar.copy if MIRROR_ENG == "a" else nc.vector.tensor_copy
            )
            for i in range(N - 1):
                mcopy(A4[:, :, i + 1 :, i], A4[:, :, i, i + 1 :])
            yield

            nc.sync.dma_start(out_r[:, offs[s] : offs[s + 1], :], A[:, :])

        pending = list(range(n_streams))
        active = [stream_gen(pending.pop(0))]
        # prime the first stream so concurrent streams stay phase-offset
        for _ in range(STAGGER):
            next(active[0])
        while pending or active:
            while len(active) < ACTIVE and pending:
                active.append(stream_gen(pending.pop(0)))
            for gen in list(active):
                try:
                    next(gen)
                except StopIteration:
                    active.remove(gen)


_CACHE = {}


def build_nc(b_core=B_CORE, n_streams=None, num_devices=N_CORES):
    if n_streams is None:
        n_streams = N_STREAMS
    key = (b_core, n_streams, num_devices)
    if key in _CACHE:
        return _CACHE[key]
    nc = bacc.Bacc(
        "TRN2", target_bir_lowering=False, debug=False, num_devices=num_devices
    )
    pos_d = nc.dram_tensor("pos", [b_core, N * D], F32, kind="ExternalInput")
    out_d = nc.dram_tensor("out", [b_core, N * N], F32, kind="ExternalOutput")
    with tile.TileContext(nc) as tc:
        emit_kernel(tc, pos_d.ap(), out_d.ap(), b_core, n_streams)
    nc.compile()
    _CACHE[key] = nc
    return nc


def run(pos_full, b_core=B_CORE, n_streams=None, n_cores=N_CORES, **kw):
    """pos_full: [n_cores*b_core, 20, 2] f32 -> [n_cores*b_core, 20, 20] f32."""
    nc = build_nc(b_core, n_streams, n_cores)
    flat = np.ascontiguousarray(
        np.asarray(pos_full, dtype=np.float32).reshape(-1, N * D)
    )
    in_maps = [
        {"pos": flat[i * b_core : (i + 1) * b_core]} for i in range(n_cores)
    ]
    res = run_bass_kernel_spmd(nc, in_maps, core_ids=list(range(n_cores)), **kw)
    out = np.concatenate([r["out"] for r in res.results], axis=0)
    return out.reshape(-1, N, N), res


def kernel(neighbor_positions, edge_list=None):
    out, _ = run(neighbor_positions)
    return out


# revision 45
# speedup vs baseline: 1.0486x; 1.0198x over previous
"""Batched 20x20 SPD covariance-matrix inversion on 8 Trainium2 NeuronCores.

For each of 131072 batches: build C = exp(-1.5 * pairwise_dist(pos)) + 0.01*I
from 20 2-D points, return C^{-1}.

Strategy (per core, data-parallel over batch):
 - batch-major layout: each of 128 SBUF partitions holds Mg matrices' full
   20x20 (400 fp32) in the free dim; N_STREAMS independent streams.
 - symmetric sweep operator (Gauss-Jordan preserving symmetry): only the
   upper triangle is updated each pivot, covered by 4 row-band rectangles.
 - engine split: the rank-1 updates (2 tensor-tensor passes per element)
   are split row-wise between DVE (tensor_tensor) and GPSIMD
   (scalar_tensor_tensor); ACT does the pivot-column gathers, the
   cov-build square/sqrt/exp, and the final negated mirror of the upper
   triangle into the lower.
 - pivot row/col are written AFTER the rank-1 update (which is allowed to
   corrupt them), so the pivot column never needs zeroing.
 - ACTIVE streams are software-pipelined round-robin at pivot granularity:
   one stream's serial pivot prefix (gather -> reciprocal -> cr scale)
   hides under the other's rank-1 work; cov builds and finalizes of
   adjacent streams overlap sweeps the same way. All pos DMAs are
   prefetched at kernel start.
"""

import numpy as np

import concourse.bass as bass  # noqa: F401  (registers engine APIs)
import concourse.tile as tile
from concourse import bacc, mybir
from concourse.bass_utils import run_bass_kernel_spmd

N = 20                  # matrix dim
D = 2                   # coord dim
PHI = 1.5
TAU = 0.01
P = 128                 # SBUF partitions
N_CORES = 8
B_TOTAL = 131072
B_CORE = B_TOTAL // N_CORES   # 16384

F32 = mybir.dt.float32
AF = mybir.ActivationFunctionType
OP = mybir.AluOpType

# Upper-triangle rectangle cover: rows [r0,r1) x cols [r0,N)
RECTS = [(0, 5), (5, 10), (10, 15), (15, 20)]

# --- engine-assignment knobs (autotuned via CoreSim) -----------------------
# per rect: how many of its rows (from the top) go to GPSIMD for the rank-1
# update; the rest go to DVE.
GP_ROWS = [5, 1, 1, 5]
# per rect: cov-build tensor-tensor ops (dx, dy, add) engine: "v" DVE, "g" GP
COV_ENG = ["v", "g", "g", "v"]
CR_ENG = "s2"            # cr = c * r:  "v" DVE tensor_mul, "g" GP stt
MIRROR_ENG = "a"        # "a" ACT copy(scale=-1) pre-negate, "v" DVE post
GATHER_SCALAR = True    # pivot-column gather on ACT (else DVE)
PIVOT_COPY_ENG = "a"    # pivot row/col <- cr copies: "v" DVE, "a" ACT, "g" GP
N_STREAMS = 8           # independent m-slices (Mg = B_CORE/P/N_STREAMS)
STREAM_SIZES = None     # optional per-stream m sizes (must sum to B_CORE/P)
ACTIVE = 2              # streams pipelined concurrently
STAGGER = 6             # yields to prime stream 0 before starting stream 1
COV_MERGED = False       # cov: one 2-coord sub (needs 2E dd tile) vs dx/dy
DIAG_ENG = "a"          # per-pivot diag<- -r + TAU add: "a" ACT, "v" DVE, "g" GP
COV_PAR_START = False    # run both initial streams' cov builds in parallel


def _gp_mul(nc, out, a, b):
    """out = a * b on GPSIMD (plain TensorTensor: >=4D APs are BIR-legal,
    unlike ScalarTensorTensor which the BIR verifier caps at 3D)."""
    nc.gpsimd.tensor_tensor(out, a, b, OP.mult)


def _gp_sub(nc, out, a, b):
    """out = a - b on GPSIMD."""
    nc.gpsimd.tensor_tensor(out, a, b, OP.subtract)


def _gp_add(nc, out, a, b):
    nc.gpsimd.tensor_tensor(out, a, b, OP.add)


def emit_kernel(tc, pos_ap, out_ap, b_core, n_streams):
    """Emit the per-core program. pos: [b_core, 40] f32, out: [b_core, 400] f32."""
    nc = tc.nc
    m_total = b_core // P
    if STREAM_SIZES is not None:
        sizes = list(STREAM_SIZES)
        n_streams = len(sizes)
    else:
        sizes = [m_total // n_streams] * n_streams
    assert sum(sizes) == m_total
    offs = [0]
    for sz in sizes:
        offs.append(offs[-1] + sz)

    pos_r = pos_ap.rearrange("(p m) f -> p m f", p=P)
    out_r = out_ap.rearrange("(p m) f -> p m f", p=P)
    big_bufs = 2 if (ACTIVE <= 2 and max(sizes) < 32) else 1

    with (
        tc.tile_pool(name="pos", bufs=1) as pos_pool,
        tc.tile_pool(name="A", bufs=big_bufs) as a_pool,
        tc.tile_pool(name="cov", bufs=big_bufs) as cov_pool,
        tc.tile_pool(name="rect", bufs=1) as rect_pool,
        tc.tile_pool(name="grect", bufs=1) as grect_pool,
        tc.tile_pool(name="small", bufs=2) as small_pool,
    ):
        # prefetch every stream's positions up front
        pos_tiles = []
        for s in range(n_streams):
            pos_t = pos_pool.tile([P, sizes[s] * N * D], F32, tag=f"pos{s}")
            nc.sync.dma_start(pos_t[:, :], pos_r[:, offs[s] : offs[s + 1], :])
            pos_tiles.append(pos_t)

        def stream_gen(s):
            par = s % ACTIVE
            Mg = sizes[s]
            posv = pos_tiles[s][:, :].rearrange(
                "p (m i d) -> p m i d", m=Mg, i=N
            )
            A = a_pool.tile([P, Mg * N * N], F32, tag=f"A{par}")
            A4 = A[:, :].rearrange("p (m i j) -> p m i j", m=Mg, i=N)

            # ---- covariance build over the upper rect cover ----
            for ri, (r0, r1) in enumerate(RECTS):
                nr, ncl = r1 - r0, N - r0
                reg = A4[:, :, r0:r1, r0:]
                sub_a = (
                    (lambda o, a, b: _gp_sub(nc, o, a, b))
                    if COV_ENG[ri] == "g"
                    else nc.vector.tensor_sub
                )
                add_a = (
                    (lambda o, a, b: _gp_add(nc, o, a, b))
                    if COV_ENG[ri] == "g"
                    else nc.vector.tensor_add
                )
                if COV_MERGED:
                    # dd[m,i,j,:] = p[i,:] - p[j,:] in ONE sub (both
                    # coords), square on ACT, strided add -> d^2 in A.
                    pi = (
                        posv[:, :, r0:r1, :]
                        .unsqueeze(3)
                        .broadcast_to([P, Mg, nr, ncl, D])
                    )
                    pj = (
                        posv[:, :, r0:, :]
                        .unsqueeze(2)
                        .broadcast_to([P, Mg, nr, ncl, D])
                    )
                    dd = cov_pool.tile(
                        [P, Mg * nr * ncl * D], F32, tag=f"dd{par}"
                    )
                    ddv = dd[:, :].rearrange(
                        "p (m i j d) -> p m i j d", m=Mg, i=nr, j=ncl
                    )
                    sub_a(ddv, pi, pj)
                    nc.scalar.square(dd[:, :], dd[:, :])
                    add_a(reg, ddv[:, :, :, :, 0], ddv[:, :, :, :, 1])
                else:
                    # dx into A (in-place square), dy in an E-sized tmp
                    xi = (
                        posv[:, :, r0:r1, 0]
                        .unsqueeze(3)
                        .broadcast_to([P, Mg, nr, ncl])
                    )
                    xj = (
                        posv[:, :, r0:, 0]
                        .unsqueeze(2)
                        .broadcast_to([P, Mg, nr, ncl])
                    )
                    yi = (
                        posv[:, :, r0:r1, 1]
                        .unsqueeze(3)
                        .broadcast_to([P, Mg, nr, ncl])
                    )
                    yj = (
                        posv[:, :, r0:, 1]
                        .unsqueeze(2)
                        .broadcast_to([P, Mg, nr, ncl])
                    )
                    dy = cov_pool.tile(
                        [P, Mg * nr * ncl], F32, tag=f"dd{par}"
                    )
                    dyv = dy[:, :].rearrange(
                        "p (m i j) -> p m i j", m=Mg, i=nr
                    )
                    sub_a(reg, xi, xj)
                    nc.scalar.square(reg, reg)
                    sub_a(dyv, yi, yj)
                    nc.scalar.square(dyv, dyv)
                    add_a(reg, reg, dyv)
                nc.scalar.sqrt(reg, reg)
                nc.scalar.activation(reg, reg, AF.Exp, scale=-PHI)
                yield

            Av = A[:, :].rearrange("p (m x) -> p m x", m=Mg)
            diag = Av[:, :, 0 : N * N : N + 1]
            if DIAG_ENG == "g":
                nc.gpsimd.tensor_scalar_add(diag, diag, TAU)
            else:
                nc.vector.tensor_scalar_add(diag, diag, TAU)

            # ---- sweep all 20 pivots (gather-free) ----
            # The raw pivot column/row is read straight out of A as
            # broadcast operands of the rank-1 muls (column k for rows
            # above the pivot, row k for rows below); cr (= c * 1/pivot)
            # is computed from A the same way. All muls are emitted
            # before any sub so the in-place subs (which corrupt the
            # pivot row/col: cr[k] = 1) never race the raw reads.
            # The reciprocal for pivot k+1 is issued right after the sub
            # that finalizes A[k+1,k+1], hiding it under pivot k's tail.
            rK = small_pool.tile([P, Mg], F32, tag=f"r{par}")
            nc.vector.reciprocal(rK[:, :], A4[:, :, 0, 0])
            for k in range(N):
                crK = small_pool.tile([P, Mg * N], F32, tag=f"cr{par}")
                cr3 = crK[:, :].rearrange("p (m i) -> p m i", m=Mg)
                rb = rK[:, :].unsqueeze(2).broadcast_to([P, Mg, N])

                def crmul(which, o, a, b):
                    # "s": col part on GPSIMD; "s2": col part alternates
                    # engines by pivot parity (finer-grained balance)
                    if which == 0 and (
                        CR_ENG == "s" or (CR_ENG == "s2" and k % 2 == 0)
                    ):
                        _gp_mul(nc, o, a, b)
                    elif CR_ENG == "g":
                        _gp_mul(nc, o, a, b)
                    else:
                        nc.vector.tensor_mul(o, a, b)

                if k:
                    crmul(0, cr3[:, :, :k], A4[:, :, :k, k], rb[:, :, :k])
                crmul(1, cr3[:, :, k:], A4[:, :, k, k:], rb[:, :, k:])

                def c_raw(a, b):
                    """Broadcast AP of raw c[a:b] read from A's storage."""
                    if b <= k + 1:  # rows at or above the pivot: column k
                        return A4[:, :, a:b, k].unsqueeze(3)
                    return A4[:, :, k, a:b].unsqueeze(3)  # below: row k

                last = k == N - 1
                subs = []
                for ri, (r0, r1) in enumerate(RECTS):
                    ncl = N - r0
                    gsplit = r0 + GP_ROWS[ri]
                    for eng, a, b in (("g", r0, gsplit), ("v", gsplit, r1)):
                        nr = b - a
                        if nr <= 0:
                            continue
                        pool = grect_pool if eng == "g" else rect_pool
                        tmp = pool.tile(
                            [P, Mg * nr * ncl], F32, tag=f"t{eng}{par}r{ri}"
                        )
                        tv = tmp[:, :].rearrange(
                            "p (m i j) -> p m i j", m=Mg, i=nr
                        )
                        crb = (
                            cr3[:, :, r0:]
                            .unsqueeze(2)
                            .broadcast_to([P, Mg, nr, ncl])
                        )
                        mul = (
                            (lambda o, x, y: _gp_mul(nc, o, x, y))
                            if eng == "g"
                            else nc.vector.tensor_mul
                        )
                        # split the band straddling the pivot: rows <= k
                        # read column k, rows > k read row k
                        if a <= k < b - 1:
                            pieces = [(a, k + 1), (k + 1, b)]
                        else:
                            pieces = [(a, b)]
                        ofs = 0
                        for (pa, pb) in pieces:
                            pn = pb - pa
                            mul(
                                tv[:, :, ofs : ofs + pn],
                                c_raw(pa, pb).broadcast_to([P, Mg, pn, ncl]),
                                crb[:, :, ofs : ofs + pn],
                            )
                            ofs += pn
                        reg = A4[:, :, a:b, r0:]
                        # does this sub finalize next pivot's diagonal?
                        owns_next = a <= k + 1 < b
                        subs.append((eng, reg, tv, owns_next))
                subs.sort(key=lambda t: not t[3])  # next-diag owner first
                for si, (eng, reg, tv, owns_next) in enumerate(subs):
                    if eng == "g":
                        if last:
                            # fold the final negation into the last
                            # pivot: reg <- tv - reg = -(reg - tv)
                            _gp_sub(nc, reg, tv, reg)
                        else:
                            _gp_sub(nc, reg, reg, tv)
                    else:
                        if last:
                            nc.vector.tensor_sub(reg, tv, reg)
                        else:
                            nc.vector.tensor_sub(reg, reg, tv)
                    if owns_next and not last:
                        rK_next = small_pool.tile(
                            [P, Mg], F32, tag=f"r{par}"
                        )
                        nc.vector.reciprocal(
                            rK_next[:, :], A4[:, :, k + 1, k + 1]
                        )
                # pivot row/col (upper parts) <- cr; diag <- -r
                # (the last pivot writes negated values: the whole rect
                # cover holds -result after its reverse subtract)
                if k < N - 1:
                    pcopy = {
                        "v": nc.vector.tensor_copy,
                        "a": nc.scalar.copy,
                        "g": nc.gpsimd.tensor_copy,
                    }[PIVOT_COPY_ENG]
                    if k:
                        pcopy(A4[:, :, :k, k], cr3[:, :, :k])
                    pcopy(A4[:, :, k, k + 1 :], cr3[:, :, k + 1 :])
                    if DIAG_ENG == "a":
                        nc.scalar.mul(A4[:, :, k, k], rK[:, :], -1.0)
                    elif DIAG_ENG == "g":
                        nc.gpsimd.tensor_scalar_mul(
                            A4[:, :, k, k], rK[:, :], -1.0
                        )
                    else:
                        nc.vector.tensor_scalar_mul(
                            A4[:, :, k, k], rK[:, :], -1.0
                        )
                else:
                    if PIVOT_COPY_ENG == "a":
                        nc.scalar.mul(A4[:, :, :k, k], cr3[:, :, :k], -1.0)
                    else:
                        nc.vector.tensor_scalar_mul(
                            A4[:, :, :k, k], cr3[:, :, :k], -1.0
                        )
                    nc.vector.tensor_copy(A4[:, :, k, k], rK[:, :])
                if k < N - 1:
                    rK = rK_next
                yield

            # ---- finalize: mirror upper -> lower (values already negated),
            # in m-halves so the first half's store overlaps the second
            # half's mirror. The last stream has no concurrent work left,
            # so its mirror runs split across the otherwise-idle DVE+GPSIMD.
            tail = s == n_streams - 1
            h = Mg // 2
            for (m0, m1) in ((0, h), (h, Mg)):
                for i in range(N - 1):
                    if tail:
                        mcopy = (
                            nc.vector.tensor_copy
                            if i % 2
                            else nc.gpsimd.tensor_copy
                        )
                    elif MIRROR_ENG == "a":
                        mcopy = nc.scalar.copy
                    else:
                        mcopy = nc.vector.tensor_copy
                    mcopy(
                        A4[:, m0:m1, i + 1 :, i], A4[:, m0:m1, i, i + 1 :]
                    )
                nc.sync.dma_start(
                    out_r[:, offs[s] + m0 : offs[s] + m1, :],
                    A[:, m0 * N * N : m1 * N * N],
                )
                yield

        pending = list(range(n_streams))
        active = [stream_gen(pending.pop(0))]
        if COV_PAR_START and pending:
            # run both initial streams' cov builds in parallel, then
            # prime stream 0's sweep so pivots stay phase-offset
            active.append(stream_gen(pending.pop(0)))
            for _ in range(len(RECTS)):
                for gen in active:
                    next(gen)
            for _ in range(STAGGER):
                next(active[0])
        else:
            # prime the first stream so concurrent streams stay offset
            for _ in range(STAGGER):
                next(active[0])
        while pending or active:
            while len(active) < ACTIVE and pending:
                active.append(stream_gen(pending.pop(0)))
            for gen in list(active):
                try:
                    next(gen)
                except StopIteration:
                    active.remove(gen)


_CACHE = {}


def build_nc(b_core=B_CORE, n_streams=None, num_devices=N_CORES):
    if n_streams is None:
        n_streams = N_STREAMS
    key = (b_core, n_streams, num_devices)
    if key in _CACHE:
        return _CACHE[key]
    nc = bacc.Bacc(
        "TRN2", target_bir_lowering=False, debug=False, num_devices=num_devices
    )
    pos_d = nc.dram_tensor("pos", [b_core, N * D], F32, kind="ExternalInput")
    out_d = nc.dram_tensor("out", [b_core, N * N], F32, kind="ExternalOutput")
    with tile.TileContext(nc) as tc:
        emit_kernel(tc, pos_d.ap(), out_d.ap(), b_core, n_streams)
    nc.compile()
    _CACHE[key] = nc
    return nc


def run(pos_full, b_core=B_CORE, n_streams=None, n_cores=N_CORES, **kw):
    """pos_full: [n_cores*b_core, 20, 2] f32 -> [n_cores*b_core, 20, 20] f32."""
    nc = build_nc(b_core, n_streams, n_cores)
    flat = np.ascontiguousarray(
        np.asarray(pos_full, dtype=np.float32).reshape(-1, N * D)
    )
    in_maps = [
        {"pos": flat[i * b_core : (i + 1) * b_core]} for i in range(n_cores)
    ]
    res = run_bass_kernel_spmd(nc, in_maps, core_ids=list(range(n_cores)), **kw)
    out = np.concatenate([r["out"] for r in res.results], axis=0)
    return out.reshape(-1, N, N), res


def kernel(neighbor_positions, edge_list=None):
    out, _ = run(neighbor_positions)
    return out
